# revision 19
# baseline (speedup 1.0000x reference)
"""TRN2 Bass kernel for nn_Attention_188978561266 (v4).

Reference computation (b=4, s=1024, d=1024, 16 heads x 64):
    qkv = x @ Wqkv ; split q,k,v
    q = q / (sqrt(mean(q^2 over ALL elements)) + eps) * scale_q   (global scalar RMS)
    k = k / (sqrt(mean(k^2 over ALL elements)) + eps) * scale_k
    attn = softmax(q @ k^T)  (no 1/sqrt(d_head), no mask)
    out = (attn @ v) @ Wo

Sharding: 8 cores = (batch b in 0..3) x (head-half in 0..1). Each core computes
qkv for its batch restricted to its 8 heads (tensor-parallel QKV columns),
full attention for those heads, and a partial (transposed) output projection.
Host sums the two partial outputs per batch and transposes. The global RMS
needs a cross-core AllReduce of two scalars (sum q^2, sum k^2).

v4 design notes:
  - All matmul operands bf16 (f32r self-loading LDWEIGHTS is ~2x slower and
    starves the exp pipeline; measured). PSUM accumulation stays fp32.
  - Phase A runs dc-outermost with 4 parallel psum accumulation chains so the
    first matmul only needs the first x chunk: A starts at DMA-arrival pace
    and the sum-sq stats (the AllReduce input) are ready ~8us earlier.
  - Phase C is paced by the Scalar-engine exp (hard floor ~80us): AV matmuls
    for head (g,i) are emitted after the S matmuls of head (g,i+1) (software
    pipeline, lag 1); the last head's S/AV/AV-prev are fully interleaved.
  - Output projection at the tail, Wo-stationary, in two 4-dout waves with
    the g0..g2 contraction emitted before the z3-dependent g3 matmuls, and
    psum->SBUF copies alternating between DVE and ACT.
  - PSUM budget: 2x psp + 2x pav tiles of [128,2,512] = 8 banks.
"""

import os as _os
import sys

sys.path.insert(0, "/opt/trn_rl_repo")

import ml_dtypes
import numpy as np

import concourse.bacc as bacc
import concourse.mybir as mybir
from concourse import library_config, tile
from concourse.bass_utils import run_bass_kernel_spmd

F32 = mybir.dt.float32
F32R = mybir.dt.float32r
BF16 = mybir.dt.bfloat16
AF = mybir.ActivationFunctionType
ALU = mybir.AluOpType
AX = mybir.AxisListType

NPBF = ml_dtypes.bfloat16

P = 128
D = 1024
S = 1024
N_HEAD = 16
DH = 64
NHL = 8          # heads per core
DC = 8           # d contraction chunks of 128
EPS = 1e-6
COUNT = 4 * 1024 * 1024   # elements of the full q (or k) tensor
N_CORES = 8
REPLICAS = [list(range(N_CORES))]

_CACHE = {}


def _build():
    nc = bacc.Bacc("TRN2", target_bir_lowering=False, debug=False, num_devices=N_CORES)

    xt = nc.dram_tensor("xt", [P, DC, S], BF16, kind="ExternalInput")
    wqk = nc.dram_tensor("wqk", [P, 8, DC, P], BF16, kind="ExternalInput")
    wv = nc.dram_tensor("wv", [P, DC, NHL * DH], BF16, kind="ExternalInput")
    wo = nc.dram_tensor("wo", [P, 4, D], BF16, kind="ExternalInput")
    qscale = nc.dram_tensor("qscale", [P, 4], F32, kind="ExternalInput")
    zpartT = nc.dram_tensor("zpartT", [D, S], F32, kind="ExternalOutput")

    with tile.TileContext(nc) as tc:
        with (
            tc.tile_pool(name="big", bufs=1) as big,
            tc.tile_pool(name="ep", bufs=3) as ep,
            tc.tile_pool(name="scr", bufs=2) as scrp,
            tc.tile_pool(name="ob", bufs=3) as obp,
            tc.tile_pool(name="small", bufs=2) as smallp,
            tc.tile_pool(name="stats", bufs=1) as stp,
            tc.tile_pool(name="ps", bufs=2, space="PSUM") as psp,
            tc.tile_pool(name="pav", bufs=2, space="PSUM") as pav,
            tc.tile_pool(name="dram", bufs=1, space="DRAM") as dramp,
        ):
            # ---- persistent SBUF tensors ----
            xT = big.tile([P, DC, S], BF16, tag="xT")
            Wqk_sb = big.tile([P, 8, DC, P], BF16, tag="Wqk")
            QT = big.tile([P, 4, S], BF16, tag="QT")
            KT = big.tile([P, 4, S], BF16, tag="KT")
            Vt = big.tile([P, 8, NHL, DH + 1], BF16, tag="Vt")
            zT = big.tile([P, 4, S], BF16, tag="zT")
            Wo_sb = big.tile([P, 4, D], BF16, tag="Wo")
            Wv_sb = big.tile([P, DC, NHL * DH], BF16, tag="Wv")

            qs_sb = stp.tile([P, 4], F32, tag="qs")
            sq_acc = stp.tile([P, 8], F32, tag="sqacc")
            qk2 = stp.tile([P, 2], F32, tag="qk2")
            g_sb = stp.tile([2, 1], F32, tag="gsb")
            gsum = stp.tile([1, 2], F32, tag="gsum")
            sc_a = stp.tile([1, 2], F32, tag="sca")
            sc_b = stp.tile([1, 2], F32, tag="scb")
            sc_c = stp.tile([1, 2], F32, tag="scc")
            pm = stp.tile([1, 1], F32, tag="pm")
            cinv = stp.tile([1, 1], F32, tag="cinv")
            c_bc = stp.tile([P, 1], F32, tag="cbc")

            dsq = stp.tile([1, 2], F32, tag="dsq")
            ones_row = stp.tile([1, P], F32, tag="ones_row")
            ones_col = stp.tile([P, 1], F32, tag="ones_col")
            ones_blk = stp.tile([P, 8, NHL, 1], F32, tag="ones_blk")

            # warm collective first: starts the CC firmware warmup / skew
            # barrier as early as possible (it does not need the library).
            cc_warm_in = dramp.tile([2, 1], F32, tag="ccwi")
            cc_warm_out = dramp.tile([2, 1], F32, tag="ccwo", addr_space="Shared")
            if _os.environ.get("KN_WARMCC", "1") == "1":
                nc.gpsimd.collective_compute(
                    "AllReduce",
                    ALU.add,
                    replica_groups=REPLICAS,
                    ins=[cc_warm_in[:]],
                    outs=[cc_warm_out[:]],
                )
            nc.gpsimd.load_library(library_config.attn)

            # ---- input DMAs + consts; wqk cts 0-3 then x chunks in arrival
            # order on two queues so the dc-outer phase A starts earliest.
            nc.sync.dma_start(Wqk_sb[:, 0, :, :], wqk[:, 0, :, :])
            nc.scalar.dma_start(Wqk_sb[:, 1, :, :], wqk[:, 1, :, :])
            nc.sync.dma_start(xT[:, 0, :], xt[:, 0, :])
            nc.scalar.dma_start(xT[:, 1, :], xt[:, 1, :])
            nc.sync.dma_start(Wqk_sb[:, 2, :, :], wqk[:, 2, :, :])
            nc.scalar.dma_start(Wqk_sb[:, 3, :, :], wqk[:, 3, :, :])
            for dc in range(2, DC):
                eng = nc.sync if dc % 2 == 0 else nc.scalar
                eng.dma_start(xT[:, dc, :], xt[:, dc, :])
            nc.sync.dma_start(qs_sb[:], qscale[:])
            for ct in range(4, 8):
                eng = nc.sync if ct % 2 == 0 else nc.scalar
                eng.dma_start(Wqk_sb[:, ct, :, :], wqk[:, ct, :, :])
            nc.scalar.dma_start(Wv_sb[:], wv[:])
            nc.vector.memset(ones_row[:], 1.0)
            nc.vector.memset(ones_col[:], 1.0)
            nc.vector.memset(ones_blk[:], 1.0)
            nc.vector.tensor_copy(Vt[:, :, :, DH : DH + 1], ones_blk[:])

            # ---- phase A: q,k projections (transposed layout) + raw sum-sq,
            # dc-outer with 4 parallel accumulation chains per half ----
            for half in range(2):
                tiles = []
                for k in range(4):
                    pool, tag = (psp, "mm2") if k < 2 else (pav, "av")
                    tiles.append(
                        pool.tile([P, 2, 512], F32, tag=tag, name=f"a_{half}_{k}")
                    )
                for dc in range(DC):
                    for k in range(4):
                        ct = 4 * half + k
                        for st in range(2):
                            nc.tensor.matmul(
                                tiles[k][:, st, :],
                                lhsT=Wqk_sb[:, ct, dc, :],
                                rhs=xT[:, dc, st * 512 : (st + 1) * 512],
                                start=(dc == 0),
                                stop=(dc == DC - 1),
                            )
                for k in range(4):
                    ct = 4 * half + k
                    ps = tiles[k]
                    scr = scrp.tile([P, 2, 512], BF16, tag="scr")
                    nc.scalar.activation(
                        scr[:], ps[:], AF.Square, accum_out=sq_acc[:, ct : ct + 1]
                    )
                    flat = ps[:].rearrange("p a b -> p (a b)")
                    if ct < 4:
                        nc.vector.tensor_scalar(
                            QT[:, ct, :], flat, qs_sb[:, ct : ct + 1], None, ALU.mult
                        )
                    else:
                        nc.vector.tensor_copy(KT[:, ct - 4, :], flat)
                if half == 0:
                    # prefetch the sqrt ACT table while the PE works on half 2
                    nc.scalar.activation(dsq[:], ones_row[:, 0:2], AF.Sqrt)

            # ---- global RMS part 1: reduce + AllReduce + scalar chain ----
            nc.vector.reduce_sum(qk2[:, 0:1], sq_acc[:, 0:4], axis=AX.X)
            nc.vector.reduce_sum(qk2[:, 1:2], sq_acc[:, 4:8], axis=AX.X)

            # One A2 chain first so the PE isn't head-of-line blocked on qk2.
            def a2_chain(sm):
                ps1 = pav.tile([P, 2, 512], F32, tag="av", name=f"a2_{sm}")
                for dc in range(DC):
                    nc.tensor.matmul(
                        ps1[:, 0, :],
                        lhsT=xT[:, dc, sm * P : (sm + 1) * P],
                        rhs=Wv_sb[:, dc, :],
                        start=(dc == 0),
                        stop=(dc == DC - 1),
                    )
                nc.vector.tensor_copy(
                    Vt[:, sm, :, 0:DH],
                    ps1[:, 0, :].rearrange("p (h d) -> p h d", h=NHL),
                )

            a2_chain(0)
            g_ps = pav.tile([P, 2, 512], F32, tag="av", name="g_ps")
            nc.tensor.matmul(
                g_ps[0:2, 0, 0:1], lhsT=qk2[:], rhs=ones_col[:], start=True, stop=True
            )
            nc.vector.tensor_copy(g_sb[:], g_ps[0:2, 0, 0:1])
            cc_in = dramp.tile([2, 1], F32, tag="ccin")
            cc_out = dramp.tile([2, 1], F32, tag="ccout", addr_space="Shared")
            nc.sync.dma_start(cc_in[:], g_sb[:])
            nc.gpsimd.collective_compute(
                "AllReduce",
                ALU.add,
                replica_groups=REPLICAS,
                ins=[cc_in[:]],
                outs=[cc_out[:]],
            )
            nc.sync.dma_start(gsum[:], cc_out[:].rearrange("a b -> b a"))
            # mean, sqrt (+1 Newton step), +eps, product, reciprocal
            nc.vector.tensor_scalar_mul(sc_a[:], gsum[:], 1.0 / COUNT)  # m
            nc.scalar.activation(sc_b[:], sc_a[:], AF.Sqrt)             # r0
            nc.vector.reciprocal(sc_c[:], sc_b[:])                      # 1/r
            nc.vector.tensor_mul(sc_c[:], sc_a[:], sc_c[:])             # m/r
            nc.vector.tensor_add(sc_b[:], sc_b[:], sc_c[:])             # r + m/r
            nc.vector.tensor_scalar(sc_b[:], sc_b[:], 0.5, EPS, ALU.mult, ALU.add)
            nc.scalar.activation(dsq[:], ones_row[:, 0:2], AF.Exp)
            nc.vector.tensor_mul(pm[:], sc_b[:, 0:1], sc_b[:, 1:2])
            nc.vector.reciprocal(cinv[:], pm[:])
            nc.gpsimd.partition_broadcast(c_bc[:], cinv[:])

            # ---- phase A2: v projection (covers the collective latency) ----
            nc.scalar.dma_start(Wo_sb[:], wo[:])
            for sm in range(1, 8):
                a2_chain(sm)

            # ---- phase C: attention, exp-paced, AV pipelined one head late --
            def emit_s(g, i, skt, E_t):
                hp = i * DH
                ps = psp.tile([P, 2, 512], F32, tag="mm2", name=f"s_{g}_{i}_{skt}")
                for jj in range(2):
                    nc.tensor.matmul(
                        ps[:, jj, :],
                        lhsT=KT[hp : hp + DH, g, skt * P : (skt + 1) * P],
                        rhs=QT[hp : hp + DH, g, jj * 512 : (jj + 1) * 512],
                        start=True,
                        stop=True,
                    )
                nc.scalar.activation(E_t[:, skt, :], ps[:], AF.Exp, scale=c_bc[:, 0:1])

            def av_mm(ps_av, g, i, skc, E_t):
                h = 2 * g + i
                for jj in range(2):
                    nc.tensor.matmul(
                        ps_av[0 : DH + 1, jj, :],
                        lhsT=Vt[:, skc, h, :],
                        rhs=E_t[:, skc, jj * 512 : (jj + 1) * 512],
                        start=(skc == 0),
                        stop=(skc == 7),
                    )

            def z_scale(ps_av, g, i, fast=False):
                # zT chunk g: low partitions = head 2g, high = head 2g+1,
                # EXCEPT chunk 3 which is swapped so the last-computed head
                # (3,1) takes the direct (no-DMA) path. Host wo layout matches.
                direct = (i == 1) if g == 3 else (i == 0)
                av_flat = ps_av[0:DH, :, :].rearrange("p a b -> p (a b)")
                rs_r = smallp.tile([1, S], F32, tag="rs", name=f"rs_{g}_{i}")
                if fast:
                    # tail fast path: per-jj halves pipelined (copy/recip on
                    # DVE overlap gpsimd broadcast of the other half).
                    assert ((i == 1) if g == 3 else (i == 0))
                    for jj in range(2):
                        sl = slice(jj * 512, (jj + 1) * 512)
                        rsh = smallp.tile(
                            [1, 512], F32, tag="rs0", name=f"rsh_{jj}"
                        )
                        nc.vector.tensor_copy(rsh[:], ps_av[DH : DH + 1, jj, :])
                        rrh = smallp.tile([1, 512], F32, tag="rs", name=f"rrh_{jj}")
                        nc.vector.reciprocal_approx_fast(rrh[:], rsh[:])
                        bch = smallp.tile(
                            [DH, 512], F32, tag="bcs", name=f"bch_{jj}"
                        )
                        nc.gpsimd.partition_broadcast(bch[:], rrh[:])
                        nc.vector.tensor_mul(
                            zT[0:DH, g, sl], ps_av[0:DH, jj, :], bch[:]
                        )
                    return
                else:
                    rs0 = smallp.tile([1, S], F32, tag="rs0", name=f"rs0_{g}_{i}")
                    nc.vector.tensor_copy(
                        rs0[:], ps_av[DH : DH + 1, :, :].rearrange("p a b -> p (a b)")
                    )
                    nc.vector.reciprocal_approx_fast(rs_r[:], rs0[:])
                    bc_sb = smallp.tile([DH, S], F32, tag="bcs", name=f"bc_{g}_{i}")
                    nc.gpsimd.partition_broadcast(bc_sb[:], rs_r[:])
                    bc_flat = bc_sb[:]
                if direct:
                    nc.vector.tensor_mul(zT[0:DH, g, :], av_flat, bc_flat)
                else:
                    ztmp = smallp.tile([DH, S], BF16, tag="ztmp", name=f"zt_{g}_{i}")
                    nc.vector.tensor_mul(ztmp[:], av_flat, bc_flat)
                    nc.sync.dma_start(zT[DH:P, g, :], ztmp[:])

            def do_av(g, i, E_t):
                ps_av = pav.tile([P, 2, 512], F32, tag="av", name=f"av_{g}_{i}")
                for skc in range(8):
                    av_mm(ps_av, g, i, skc, E_t)
                z_scale(ps_av, g, i)

            iters = [(g, i) for g in range(4) for i in range(2)]
            prev = None
            n_warm = int(_os.environ.get("KN_WARM", "64"))
            for g, i in iters[:-1]:
                E_t = ep.tile([P, 8, S], BF16, tag="E", name=f"E_{g}_{i}")
                for skt in range(8):
                    emit_s(g, i, skt, E_t)
                    if (g, i) == (0, 0) and skt == 1 and n_warm:
                        # PE warm-keepers: the first two S psums are banked
                        # for exp; these dummies keep the PE clock-gate at
                        # full speed across the AllReduce wait so phase C
                        # starts (and stays) at the fast p-state.
                        warm_ps = pav.tile(
                            [P, 2, 512], F32, tag="av", name="warm_ps"
                        )
                        for _ in range(n_warm):
                            nc.tensor.matmul(
                                warm_ps[:, 0, :],
                                lhsT=KT[0:DH, 0, 0:P],
                                rhs=QT[0:DH, 0, 0:512],
                                start=True,
                                stop=True,
                            )
                if prev is not None:
                    do_av(*prev)
                prev = (g, i, E_t)

            # last head (3,1): interleave its S matmuls with the lagged
            # AV(3,0) and its own AV so the post-exp drain is minimal.
            g, i = 3, 1
            E_t = ep.tile([P, 8, S], BF16, tag="E", name="E_3_1")
            E_prev = prev[2]
            ps_av_a = pav.tile([P, 2, 512], F32, tag="av", name="av_3_0")
            ps_av_b = pav.tile([P, 2, 512], F32, tag="av", name="av_3_1")
            for skt in range(8):
                emit_s(g, i, skt, E_t)
                av_mm(ps_av_a, 3, 0, skt, E_prev)
                if skt >= 2:
                    av_mm(ps_av_b, 3, 1, skt - 2, E_t)
            z_scale(ps_av_a, 3, 0)
            for skc in (6, 7):
                av_mm(ps_av_b, 3, 1, skc, E_t)
            z_scale(ps_av_b, 3, 1, fast=True)

            # ---- phase D: output projection at the tail, two 4-dout waves.
            # g0..g2 contraction first (z chunks 0-2 long ready), then the
            # z3-gated g3 matmuls; copies alternate DVE / ACT.
            for wave in range(2):
                douts = list(range(4 * wave, 4 * wave + 4))
                tiles = {}
                for do_ in douts:
                    pool, tag = (psp, "mm2") if do_ % 4 < 2 else (pav, "av")
                    tiles[do_] = pool.tile(
                        [P, 2, 512], F32, tag=tag, name=f"o_{do_}"
                    )
                for gg in range(4):
                    for do_ in douts:
                        for nt in range(2):
                            nc.tensor.matmul(
                                tiles[do_][:, nt, :],
                                lhsT=Wo_sb[:, gg, do_ * P : (do_ + 1) * P],
                                rhs=zT[:, gg, nt * 512 : (nt + 1) * 512],
                                start=(gg == 0),
                                stop=(gg == 3),
                            )
                for do_ in douts:
                    ob = obp.tile([P, 2, 512], F32, tag="ob", name=f"ob_{do_}")
                    if do_ % 2:
                        nc.vector.tensor_copy(ob[:], tiles[do_][:])
                    else:
                        nc.scalar.activation(ob[:], tiles[do_][:], AF.Copy)
                    nc.sync.dma_start(
                        zpartT[do_ * P : (do_ + 1) * P, :],
                        ob[:].rearrange("p a b -> p (a b)"),
                    )

    nc.compile()
    return nc


def _get_nc():
    if "nc" not in _CACHE:
        _CACHE["nc"] = _build()
    return _CACHE["nc"]


def _prep_core_inputs(x, Wqkv, Wo, scale_q, scale_k):
    """Host-side shard + layout prep. Returns list of 8 in_maps."""
    x = np.asarray(x, dtype=np.float32)
    Wqkv = np.asarray(Wqkv, dtype=np.float32)
    Wo = np.asarray(Wo, dtype=np.float32)
    scale_q = np.asarray(scale_q, dtype=np.float32)
    scale_k = np.asarray(scale_k, dtype=np.float32)

    # combined per-d_head scale folded into Q (applied at the psum->SBUF copy)
    qs_vec = np.tile(scale_q * scale_k, NHL)               # [512]
    qs_dev = np.ascontiguousarray(qs_vec.reshape(4, P).T)  # [128,4]

    xt_all = []
    for b in range(4):
        xTb = x[b].T                                       # [d, s]
        xt_all.append(
            np.ascontiguousarray(
                xTb.reshape(DC, P, S).transpose(1, 0, 2).astype(NPBF)
            )
        )  # [128, 8, 1024]

    in_maps = []
    for c in range(8):
        b = c // 2
        hh = (c % 2) * NHL
        cols = slice(hh * DH, (hh + NHL) * DH)
        wq_c = Wqkv[:, 0 * D:1 * D][:, cols]               # [1024, 512]
        wk_c = Wqkv[:, 1 * D:2 * D][:, cols]
        wv_c = Wqkv[:, 2 * D:3 * D][:, cols]
        wqk_c = np.concatenate([wq_c, wk_c], axis=1)       # [1024, 1024]
        # [p, ct, dc, n]: per-ct slices are contiguous per-partition DMAs
        wqk_dev = np.ascontiguousarray(
            wqk_c.reshape(DC, P, 8, P).transpose(1, 2, 0, 3).astype(NPBF)
        )
        wv_dev = np.ascontiguousarray(
            wv_c.reshape(DC, P, NHL * DH).transpose(1, 0, 2).astype(NPBF)
        )
        # Wo rows for local heads, arranged [128, 4, 1024]:
        # chunk g low half = local head 2g, high half = local head 2g+1
        # (matches the zT packing of head pairs on partition halves)
        wo_loc = Wo[(hh * DH):(hh + NHL) * DH, :]          # [512, 1024]
        wo_dev = np.empty((P, 4, D), dtype=np.float32)
        for g in range(4):
            lo, hi = 2 * g, 2 * g + 1
            if g == 3:
                lo, hi = hi, lo  # chunk 3 head order swapped (see z_scale)
            wo_dev[0:DH, g, :] = wo_loc[lo * DH:(lo + 1) * DH, :]
            wo_dev[DH:P, g, :] = wo_loc[hi * DH:(hi + 1) * DH, :]
        in_maps.append(
            {
                "xt": xt_all[b],
                "wqk": wqk_dev,
                "wv": wv_dev,
                "wo": np.ascontiguousarray(wo_dev.astype(NPBF)),
                "qscale": qs_dev,
            }
        )
    return in_maps


def run(x, Wqkv, Wo, scale_q, scale_k, trace=False):
    nc = _get_nc()
    in_maps = _prep_core_inputs(x, Wqkv, Wo, scale_q, scale_k)
    res = run_bass_kernel_spmd(
        nc, in_maps, core_ids=list(range(N_CORES)), trace=trace
    )
    out = np.empty((4, S, D), dtype=np.float32)
    for b in range(4):
        zt = res.results[2 * b]["zpartT"] + res.results[2 * b + 1]["zpartT"]
        out[b] = zt.T
    return out, res


def kernel(x, Wqkv, Wo, scale_q, scale_k):
    out, _ = run(x, Wqkv, Wo, scale_q, scale_k, trace=False)
    return out
